# revision 34
# baseline (speedup 1.0000x reference)
"""Exponentiated-quadratic (RBF) kernel matrix on 8 Trainium2 NeuronCores.

K[i, j] = sigma * exp(-0.5 * ||x1_i/rho - x2_j/rho||^2)
        with sigma = exp(log_sigma)^2, rho = exp(log_rho)

Strategy
--------
Row-shard x1 across the 8 cores (512 rows each), replicate x2. The squared
distance folds into a single augmented matmul: with
  a_i = [x1_i/rho, -0.5*||x1_i/rho||^2, 1]          (P+2 = 34 wide)
  b_j = [x2_j/rho, 1, -0.5*||x2_j/rho||^2]
the dot product a_i . b_j = -0.5 * d_ij, so the whole epilogue is one
ScalarE activation: K = exp(s + 2*log_sigma)  (ACT computes func(x*scale+bias)
for free). Host prep is only the tiny (N,P) scaling/transpose; all O(N*M)
work (matmul, exp, output traffic) runs on-device.

The matmul runs as a 3-pass bf16 split (s = Ah.Bh + Ah.Bl + Al.Bh with
A = Ah + Al exactly in bf16) accumulated in fp32 PSUM: bf16 streams the PE
at 1 column/cycle (fp32 is 4x slower, fp32r is TF32-precision) and the
dropped Al.Bl term is ~2^-18 relative — measured 1.9e-5 scale-relative
output error.

Per core: 4 row-blocks of 128 x 4096. Each row-block is 2 PSUM tiles
(128 x 2048 = 4 banks); each PSUM tile takes 12 matmuls (3 passes x 4
column-slices of 512), one exp-activation PSUM->SBUF, and the row-block
ends with one 2 MiB DMA to HBM. Output-DMA roofline: 8 MiB/core at
~360 GB/s ~= 23 us.

walrus in this container rejects instructions carrying more than one
semaphore wait, which shapes three things: the first B chunk is fused into
the same DRAM tensor/DMA as A (first matmul = one wait), PSUM tiles are
persistent (pool re-allocation adds a same-engine PE wait), and a chain of
single-wait NOPs on the sync sequencer "observes" every completion before
the framework's kernel-tail drain (which otherwise waits on all ~10 sems
at once).
"""

import numpy as np
import ml_dtypes

import concourse.bass as bass
import concourse.mybir as mybir
import concourse.tile as tile
from concourse.bass_utils import run_bass_kernel_spmd
from concourse.tile import add_dep_helper

N, M, P = 4096, 4096, 32
NCORES = 8
NSHARD = N // NCORES  # 512 rows of x1 per core
KAUG = P + 2          # 34: contraction dim after augmentation
IBLK = 128            # output row-block = PSUM partition dim
JBLK = 512            # matmul free dim = one fp32 PSUM bank
PSW = 2048            # PSUM tile width (4 banks) = one exp-activation
BCH = 2048            # B chunk width (pipelines input load under compute)

BF16 = mybir.dt.bfloat16
NPBF16 = ml_dtypes.bfloat16
STRIP = 64            # PE row-strip stride for 2-way row packing (K=34 -> 64)
PAD = 64              # extra DRAM columns per input row (see _build_nc)


def _build_nc(bias_val: float):
    nc = bass.Bass()
    # in1 = [A_hi | A_lo | B_hi[:, :BCH] | B_lo[:, :BCH]], in2 the remaining
    # B chunks. Each is loaded twice (row strips 0 and 1) = 4 loads + 4
    # stores = 8 DMAs, one per HWDGE lane (lane reuse adds an ordering wait
    # that walrus rejects). DRAM rows carry PAD extra columns so the source
    # access pattern stays strided: a fully contiguous source collapses into
    # a handful of descriptors served by 1-2 SDMA engines (~46 GB/s); the
    # strided form emits one descriptor per row and engages all engines
    # owning the target partitions.
    in1_w = 2 * NSHARD + 2 * BCH
    in2_w = 2 * (M - BCH)
    in1_t = nc.declare_dram_parameter(
        "in1_t", [KAUG, in1_w + PAD], BF16, isOutput=False
    )
    in2_t = nc.declare_dram_parameter(
        "in2_t", [KAUG, in2_w + PAD], BF16, isOutput=False
    )
    out = nc.declare_dram_parameter("out", [NSHARD, M], mybir.dt.float32, isOutput=True)

    with tile.TileContext(nc) as tc:
        with (
            tc.tile_pool(name="inp", bufs=1) as inp_pool,
            tc.tile_pool(name="stage", bufs=1) as stage_pool,
            tc.tile_pool(name="ps", bufs=1, space="PSUM") as ps_pool,
        ):
            # Inputs load into partitions 0..33 (row strip 0) and 64..97
            # (row strip 1) — the two strips live on disjoint SBUF port
            # groups, so the loads run fully parallel. Matmuls alternate
            # strips via tile_position so two streams run concurrently in
            # the PE array and each strip's LDWEIGHTS prefetches under the
            # other strip's matmul.
            dma_insts = []
            in1_sb = inp_pool.tile([2 * STRIP, in1_w], BF16, tag="in1")
            in2_sb = inp_pool.tile([2 * STRIP, in2_w], BF16, tag="in2")
            for sb, dram, w in (
                (in1_sb, in1_t, in1_w),
                (in2_sb, in2_t, in2_w),
            ):
                for s in range(2):
                    r = slice(s * STRIP, s * STRIP + KAUG)
                    dma_insts.append(
                        nc.sync.dma_start(out=sb[r, :], in_=dram[:, :w])
                    )

            def strip_aps(s):
                r = slice(s * STRIP, s * STRIP + KAUG)
                return (
                    in1_sb[r, 0:NSHARD],                      # a_hi
                    in1_sb[r, NSHARD : 2 * NSHARD],           # a_lo
                    [
                        in1_sb[r, 2 * NSHARD : 2 * NSHARD + BCH],
                        in2_sb[r, 0 : M - BCH],
                    ],
                    [
                        in1_sb[r, 2 * NSHARD + BCH : 2 * NSHARD + 2 * BCH],
                        in2_sb[r, M - BCH : 2 * (M - BCH)],
                    ],
                )

            strips = [strip_aps(0), strip_aps(1)]

            # Persistent PSUM tiles (see module docstring).
            ps_tiles = [
                ps_pool.tile(
                    [IBLK, PSW], mybir.dt.float32, tag=f"ps{h}", name=f"ps{h}"
                )
                for h in range(M // PSW)
            ]

            act_insts = []
            mm_insts = []
            for i in range(NSHARD // IBLK):  # 4 row-blocks
                out_sb = stage_pool.tile(
                    [IBLK, M], mybir.dt.float32, tag=f"out{i}", name=f"out{i}"
                )
                for h in range(M // PSW):  # 2 PSUM tiles per row-block
                    ps = ps_tiles[h]
                    nq = PSW // JBLK  # 4 column slices
                    # 3 passes; column slice q runs in row strip q%2.
                    for pass_idx, (start, stop) in enumerate(
                        ((True, False), (False, False), (False, True))
                    ):
                        for q in range(nq):
                            s = q % 2
                            a_hi, a_lo, bh_chunks, bl_chunks = strips[s]
                            lhsT = (a_hi, a_hi, a_lo)[pass_idx][
                                :, i * IBLK : (i + 1) * IBLK
                            ]
                            rhs_ch = (bh_chunks, bl_chunks, bh_chunks)[
                                pass_idx
                            ][h]
                            mm_insts.append(
                                nc.tensor.matmul(
                                    ps[:, q * JBLK : (q + 1) * JBLK],
                                    lhsT=lhsT,
                                    rhs=rhs_ch[:, q * JBLK : (q + 1) * JBLK],
                                    start=start,
                                    stop=stop,
                                    tile_position=(s * STRIP, 0),
                                )
                            )
                    act_insts.append(
                        nc.scalar.activation(
                            out=out_sb[:, h * PSW : (h + 1) * PSW],
                            in_=ps,
                            func=mybir.ActivationFunctionType.Exp,
                            bias=float(bias_val),
                            scale=1.0,
                        )
                    )
                dma_insts.append(
                    nc.sync.dma_start(
                        out=out[i * IBLK : (i + 1) * IBLK, :], in_=out_sb
                    )
                )

            # ACT->ACT sync deps come from PSUM bank-pair serialization of two
            # READS of the same tile — already transitively ordered through
            # the interleaved matmuls, and same-engine FIFO besides. Demote to
            # nosync (ordering-only): walrus rejects ACTIVATE instructions
            # carrying more than one semaphore wait.
            import bass_rust as _br

            act_names = {a.ins.name for a in act_insts}
            for a in act_insts:
                deps = list(a.ins.sync_dependency_names())
                spurious = [d for d in deps if d in act_names]
                if spurious:
                    keep = [d for d in deps if d not in act_names]
                    a.ins.take_sync_dependencies()
                    a.ins.set_sync_dependencies(
                        _br.InstructionNameOrderedSet(keep)
                    )
                    a.ins.add_nosync_dependencies_from(
                        _br.InstructionNameOrderedSet(spurious)
                    )

            # Wait-funnel for the kernel-tail drain: the framework drain waits
            # on every live semaphore at once, which walrus rejects (sync-wait
            # slot limit). Observe each completion on the SP sequencer via
            # single-wait nops first, so the drain itself needs no waits.
            for t in [mm_insts[-1], act_insts[-1], *dma_insts]:
                nop = nc.sync.nop(nofuse=True, hint="tail_funnel")
                add_dep_helper(nop.ins, t.ins, True, "tail wait funnel")
    return nc


def run(x1, x2, log_rho, log_sigma, trace=False):
    """Returns (K, exec_time_ns). exec_time_ns is None unless trace=True."""
    x1 = np.asarray(x1, dtype=np.float32)
    x2 = np.asarray(x2, dtype=np.float32)
    rho = float(np.exp(np.float64(np.asarray(log_rho))))
    bias = 2.0 * float(np.asarray(log_sigma))  # log(sigma) = 2*log_sigma

    xs = (x1 / np.float32(rho)).astype(np.float32)
    ys = (x2 / np.float32(rho)).astype(np.float32)
    xn = np.einsum("np,np->n", xs, xs, dtype=np.float64)
    yn = np.einsum("mp,mp->m", ys, ys, dtype=np.float64)

    a_full = np.empty((KAUG, N), np.float32)
    a_full[:P] = xs.T
    a_full[P] = (-0.5 * xn).astype(np.float32)
    a_full[P + 1] = 1.0
    b_full = np.empty((KAUG, M), np.float32)
    b_full[:P] = ys.T
    b_full[P] = 1.0
    b_full[P + 1] = (-0.5 * yn).astype(np.float32)

    a_hi = a_full.astype(NPBF16)
    a_lo = (a_full - a_hi.astype(np.float32)).astype(NPBF16)
    b_hi = b_full.astype(NPBF16)
    b_lo = (b_full - b_hi.astype(np.float32)).astype(NPBF16)

    pad = np.zeros((KAUG, PAD), NPBF16)
    in2 = np.ascontiguousarray(
        np.concatenate([b_hi[:, BCH:], b_lo[:, BCH:], pad], axis=1)
    )

    nc = _build_nc(bias)
    in_maps = []
    for c in range(NCORES):
        sl = slice(c * NSHARD, (c + 1) * NSHARD)
        in1 = np.concatenate(
            [a_hi[:, sl], a_lo[:, sl], b_hi[:, :BCH], b_lo[:, :BCH], pad],
            axis=1,
        )
        in_maps.append(
            {"in1_t": np.ascontiguousarray(in1), "in2_t": in2}
        )
    res = run_bass_kernel_spmd(
        nc, in_maps, core_ids=list(range(NCORES)), trace=trace
    )
    full = np.concatenate(
        [res.results[c]["out"] for c in range(NCORES)], axis=0
    )
    return full, res.exec_time_ns


def kernel(x1, x2, log_rho, log_sigma):
    out, _ = run(x1, x2, log_rho, log_sigma, trace=False)
    return out


# revision 36
# speedup vs baseline: 1.4340x; 1.4340x over previous
"""Exponentiated-quadratic (RBF) kernel matrix on 8 Trainium2 NeuronCores.

K[i, j] = sigma * exp(-0.5 * ||x1_i/rho - x2_j/rho||^2)
        with sigma = exp(log_sigma)^2, rho = exp(log_rho)

Strategy
--------
Row-shard x1 across the 8 cores (512 rows each), replicate x2. Each core
computes S = (x1/rho) @ (x2/rho)^T - 0.5*||y_j||^2 on the tensor engine and
finishes with one ScalarE activation per PSUM tile:
K = exp(S + (-0.5*||x_i||^2 + 2*log_sigma)), using ACT's free per-partition
bias (exact fp32 for the x-norms) — so the whole epilogue is a single pass.

Matmul precision: 3-pass bf16 split (Ah.Bh + Ah.Bl + Al.Bh, fp32 PSUM
accumulation, the dropped Al.Bl term is ~2^-18 relative) plus a K=3
ones-weighted pass adding the triple-bf16-split -0.5*||y_j||^2 row. Measured
~2e-5 scale-relative output error.

PE utilisation: K=32 fits a 32-row strip of the 128x128 array, so four
matmul streams run CONCURRENTLY via tile_position=(32s, 0) — column slice q
of each PSUM tile runs in strip q. This quadruples matmul throughput and
lets each strip's LDWEIGHTS prefetch under the other strips' matmuls.

The 4-strip layout also makes every input DMA a dense 128-partition
transfer (the fast path: ~300 GB/s vs ~50 GB/s for a 34-partition load): B
is packed on the host so strip s's rows hold exactly the columns strip s
consumes (no duplication), A is replicated per strip, and the fp32 ACT bias
rides along bit-cast as bf16 column pairs. Two input loads go on the two
parallel HWDGE rings (sync + scalar); output stores alternate between the
rings as well.

walrus in this container rejects instructions carrying more than one
semaphore wait, which shapes several things: single fused input tensors
(first matmul = one wait), persistent PSUM tiles (pool re-allocation adds a
same-engine PE wait), total DMA count <= 8 (HWDGE lane reuse adds an
ordering wait), ACT->ACT pseudo-deps demoted to nosync, and a chain of
single-wait NOPs on the sync sequencer that "observes" every completion
before the framework's kernel-tail drain.
"""

import numpy as np
import ml_dtypes

import concourse.bass as bass
import concourse.mybir as mybir
import concourse.tile as tile
from concourse.bass_utils import run_bass_kernel_spmd
from concourse.tile import add_dep_helper

N, M, P = 4096, 4096, 32
NCORES = 8
NSHARD = N // NCORES  # 512 rows of x1 per core
IBLK = 128            # output row-block = PSUM partition dim
JBLK = 512            # matmul free dim = one fp32 PSUM bank
PSW = 2048            # PSUM tile width (4 banks) = one exp-activation
NSTRIP = 4            # concurrent PE row strips (K=32 each)
NI = NSHARD // IBLK   # 4 row-blocks
NH = M // PSW         # 2 PSUM tiles per row-block

BF16 = mybir.dt.bfloat16
NPBF16 = ml_dtypes.bfloat16

# load1 column layout (all bf16, 128 partitions):
#   [A_hi 512 | A_lo 512 | Bh(h=0) 512 | Bl(h=0) 512 | Yn(h=0) 512 |
#    ones 128 | xn_bits 8 | pad 56]                           -> 2752 cols
# load2: [Bh(h=1) 512 | Bl(h=1) 512 | Yn(h=1) 512 | pad 64]   -> 1600 cols
AHI_O = 0
ALO_O = 512
BH0_O = 1024
BL0_O = 1536
YN0_O = 2048
ONES_O = 2560
XN_O = 2688
L1_W = 2752
L2_W = 1600


def _build_nc():
    nc = bass.Bass()
    l1_t = nc.declare_dram_parameter("l1_t", [IBLK, L1_W], BF16, isOutput=False)
    l2_t = nc.declare_dram_parameter("l2_t", [IBLK, L2_W], BF16, isOutput=False)
    out = nc.declare_dram_parameter("out", [NSHARD, M], mybir.dt.float32, isOutput=True)

    with tile.TileContext(nc) as tc:
        with (
            tc.tile_pool(name="inp", bufs=1) as inp_pool,
            tc.tile_pool(name="stage", bufs=1) as stage_pool,
            tc.tile_pool(name="ps", bufs=1, space="PSUM") as ps_pool,
        ):
            dma_insts = []
            l1_sb = inp_pool.tile([IBLK, L1_W], BF16, tag="l1")
            dma_insts.append(nc.sync.dma_start(out=l1_sb, in_=l1_t[:, :]))
            l2_sb = inp_pool.tile([IBLK, L2_W], BF16, tag="l2")
            dma_insts.append(nc.scalar.dma_start(out=l2_sb, in_=l2_t[:, :]))

            def rows(s, k=32):
                return slice(32 * s, 32 * s + k)

            def bh(h, s):
                sb, o = (l1_sb, BH0_O) if h == 0 else (l2_sb, 0)
                return sb[rows(s), o : o + JBLK]

            def bl(h, s):
                sb, o = (l1_sb, BL0_O) if h == 0 else (l2_sb, JBLK)
                return sb[rows(s), o : o + JBLK]

            def ynr(h, s):
                sb, o = (l1_sb, YN0_O) if h == 0 else (l2_sb, 2 * JBLK)
                return sb[rows(s, 3), o : o + JBLK]

            xn_bias = l1_sb[:, XN_O : XN_O + 2 * NI].bitcast(mybir.dt.float32)

            # Tiny ACT-engine read of l1 so the scalar engine observes the l1
            # DMA semaphore here (1 wait); the real activations then carry
            # only their PE wait (walrus rejects multi-wait ACTIVATE, and
            # Tile doesn't track that the PE wait transitively covers l1).
            scratch = inp_pool.tile([IBLK, 1], mybir.dt.float32, tag="scr")
            nc.scalar.copy(out=scratch, in_=l1_sb[:, 0:1])

            ps_tiles = [
                ps_pool.tile(
                    [IBLK, PSW], mybir.dt.float32, tag=f"ps{h}", name=f"ps{h}"
                )
                for h in range(NH)
            ]

            act_insts = []
            mm_insts = []
            for i in range(NI):
                out_sb = stage_pool.tile(
                    [IBLK, M], mybir.dt.float32, tag=f"out{i}", name=f"out{i}"
                )
                for h in range(NH):
                    ps = ps_tiles[h]
                    # 4 passes x 4 strips; strip s = column slice q=s of the
                    # PSUM tile. Inner loop cycles strips so consecutive
                    # matmuls run in different row groups (concurrent).
                    for p in range(4):
                        start = p == 0
                        stop = p == 3
                        for s in range(NSTRIP):
                            if p == 0:
                                lhsT = l1_sb[rows(s), AHI_O + i * IBLK : AHI_O + (i + 1) * IBLK]
                                rhs = bh(h, s)
                            elif p == 1:
                                lhsT = l1_sb[rows(s), AHI_O + i * IBLK : AHI_O + (i + 1) * IBLK]
                                rhs = bl(h, s)
                            elif p == 2:
                                lhsT = l1_sb[rows(s), ALO_O + i * IBLK : ALO_O + (i + 1) * IBLK]
                                rhs = bh(h, s)
                            else:
                                lhsT = l1_sb[rows(s, 3), ONES_O : ONES_O + IBLK]
                                rhs = ynr(h, s)
                            mm_insts.append(
                                nc.tensor.matmul(
                                    ps[:, s * JBLK : (s + 1) * JBLK],
                                    lhsT=lhsT,
                                    rhs=rhs,
                                    start=start,
                                    stop=stop,
                                    tile_position=(32 * s, 0),
                                )
                            )
                    act_insts.append(
                        nc.scalar.activation(
                            out=out_sb[:, h * PSW : (h + 1) * PSW],
                            in_=ps,
                            func=mybir.ActivationFunctionType.Exp,
                            bias=xn_bias[:, i : i + 1],
                            scale=1.0,
                        )
                    )
                eng = nc.sync if i % 2 == 0 else nc.scalar
                dma_insts.append(
                    eng.dma_start(
                        out=out[i * IBLK : (i + 1) * IBLK, :], in_=out_sb
                    )
                )

            # Demote ACT->ACT pseudo-deps (PSUM bank read-read serialization,
            # already ordered through the interleaved matmuls + same-engine
            # FIFO) to nosync: walrus rejects multi-wait ACTIVATE.
            import bass_rust as _br

            act_names = {a.ins.name for a in act_insts}
            for a in act_insts:
                deps = list(a.ins.sync_dependency_names())
                spurious = [d for d in deps if d in act_names]
                if spurious:
                    keep = [d for d in deps if d not in act_names]
                    a.ins.take_sync_dependencies()
                    a.ins.set_sync_dependencies(
                        _br.InstructionNameOrderedSet(keep)
                    )
                    a.ins.add_nosync_dependencies_from(
                        _br.InstructionNameOrderedSet(spurious)
                    )

            # Wait-funnel so the framework's kernel-tail drain needs no waits
            # of its own (walrus rejects its usual all-sems wait list).
            for t in [mm_insts[-1], act_insts[-1], *dma_insts]:
                nop = nc.sync.nop(nofuse=True, hint="tail_funnel")
                add_dep_helper(nop.ins, t.ins, True, "tail wait funnel")
    return nc


def _bf16_splits(x, n):
    """Split fp32 array into n bf16 parts summing to ~x."""
    parts = []
    rem = x.astype(np.float32)
    for _ in range(n):
        p = rem.astype(NPBF16)
        parts.append(p)
        rem = rem - p.astype(np.float32)
    return parts


def run(x1, x2, log_rho, log_sigma, trace=False):
    """Returns (K, exec_time_ns). exec_time_ns is None unless trace=True."""
    x1 = np.asarray(x1, dtype=np.float32)
    x2 = np.asarray(x2, dtype=np.float32)
    rho = float(np.exp(np.float64(np.asarray(log_rho))))
    log_sig = 2.0 * float(np.asarray(log_sigma))  # log(sigma)

    xs = (x1 / np.float32(rho)).astype(np.float32)
    ys = (x2 / np.float32(rho)).astype(np.float32)
    xn = np.einsum("np,np->n", xs, xs, dtype=np.float64)
    yn = np.einsum("mp,mp->m", ys, ys, dtype=np.float64)

    a = xs.T.astype(np.float32)  # (32, N)
    b = ys.T.astype(np.float32)  # (32, M)
    a_hi, a_lo = _bf16_splits(a, 2)
    b_hi, b_lo = _bf16_splits(b, 2)
    y1, y2, y3 = _bf16_splits((-0.5 * yn).astype(np.float32), 3)
    # per-row ACT bias: -0.5*||x_i||^2 + log(sigma), exact fp32
    xbias = ((-0.5 * xn) + log_sig).astype(np.float32)

    def pack_b(src, h):
        # strip s rows hold the columns strip s consumes: B[:, h*PSW+s*JBLK..]
        o = np.zeros((IBLK, JBLK), NPBF16)
        for s in range(NSTRIP):
            o[32 * s : 32 * s + 32] = src[:, h * PSW + s * JBLK : h * PSW + (s + 1) * JBLK]
        return o

    def pack_yn(h):
        o = np.zeros((IBLK, JBLK), NPBF16)
        for s in range(NSTRIP):
            for r, yr in enumerate((y1, y2, y3)):
                o[32 * s + r] = yr[h * PSW + s * JBLK : h * PSW + (s + 1) * JBLK]
        return o

    ones = np.zeros((IBLK, IBLK), NPBF16)
    for s in range(NSTRIP):
        ones[32 * s : 32 * s + 3] = NPBF16(1.0)

    l2 = np.zeros((IBLK, L2_W), NPBF16)
    l2[:, 0:JBLK] = pack_b(b_hi, 1)
    l2[:, JBLK : 2 * JBLK] = pack_b(b_lo, 1)
    l2[:, 2 * JBLK : 3 * JBLK] = pack_yn(1)

    nc = _build_nc()
    in_maps = []
    for c in range(NCORES):
        sl = slice(c * NSHARD, (c + 1) * NSHARD)
        l1 = np.zeros((IBLK, L1_W), NPBF16)
        for s in range(NSTRIP):
            l1[32 * s : 32 * s + 32, AHI_O : AHI_O + NSHARD] = a_hi[:, sl]
            l1[32 * s : 32 * s + 32, ALO_O : ALO_O + NSHARD] = a_lo[:, sl]
        l1[:, BH0_O : BH0_O + JBLK] = pack_b(b_hi, 0)
        l1[:, BL0_O : BL0_O + JBLK] = pack_b(b_lo, 0)
        l1[:, YN0_O : YN0_O + JBLK] = pack_yn(0)
        l1[:, ONES_O : ONES_O + IBLK] = ones
        # fp32 bias bits ride along as bf16 column pairs
        xb = np.zeros((IBLK, NI), np.float32)
        for i in range(NI):
            xb[:, i] = xbias[c * NSHARD + i * IBLK : c * NSHARD + (i + 1) * IBLK]
        l1[:, XN_O : XN_O + 2 * NI] = xb.view(np.uint16).view(NPBF16)
        in_maps.append({"l1_t": np.ascontiguousarray(l1), "l2_t": l2})

    res = run_bass_kernel_spmd(
        nc, in_maps, core_ids=list(range(NCORES)), trace=trace
    )
    full = np.concatenate(
        [res.results[c]["out"] for c in range(NCORES)], axis=0
    )
    return full, res.exec_time_ns


def kernel(x1, x2, log_rho, log_sigma):
    out, _ = run(x1, x2, log_rho, log_sigma, trace=False)
    return out


# revision 37
# speedup vs baseline: 1.4755x; 1.0289x over previous
"""Exponentiated-quadratic (RBF) kernel matrix on 8 Trainium2 NeuronCores.

K[i, j] = sigma * exp(-0.5 * ||x1_i/rho - x2_j/rho||^2)
        with sigma = exp(log_sigma)^2, rho = exp(log_rho)

Strategy
--------
Row-shard x1 across the 8 cores (512 rows each), replicate x2. Each core
computes S = (x1/rho) @ (x2/rho)^T - 0.5*||y_j||^2 on the tensor engine and
finishes with one ScalarE activation per PSUM tile:
K = exp(S + (-0.5*||x_i||^2 + 2*log_sigma)), using ACT's free per-partition
bias (exact fp32 for the x-norms) — so the whole epilogue is a single pass.

Matmul precision: 3-pass bf16 split (Ah.Bh + Ah.Bl + Al.Bh, fp32 PSUM
accumulation, the dropped Al.Bl term is ~2^-18 relative) plus a K=3
ones-weighted pass adding the triple-bf16-split -0.5*||y_j||^2 row. Measured
~2e-5 scale-relative output error.

PE utilisation: K=32 fits a 32-row strip of the 128x128 array, so four
matmul streams run CONCURRENTLY via tile_position=(32s, 0) — column slice q
of each PSUM tile runs in strip q. This quadruples matmul throughput and
lets each strip's LDWEIGHTS prefetch under the other strips' matmuls.

The 4-strip layout also makes every input DMA a dense 128-partition
transfer (the fast path: ~300 GB/s vs ~50 GB/s for a 34-partition load): B
is packed on the host so strip s's rows hold exactly the columns strip s
consumes (no duplication), A is replicated per strip, and the fp32 ACT bias
rides along bit-cast as bf16 column pairs. Two input loads go on the two
parallel HWDGE rings (sync + scalar); output stores alternate between the
rings as well.

walrus in this container rejects instructions carrying more than one
semaphore wait, which shapes several things: single fused input tensors
(first matmul = one wait), persistent PSUM tiles (pool re-allocation adds a
same-engine PE wait), total DMA count <= 8 (HWDGE lane reuse adds an
ordering wait), ACT->ACT pseudo-deps demoted to nosync, and a chain of
single-wait NOPs on the sync sequencer that "observes" every completion
before the framework's kernel-tail drain.
"""

import numpy as np
import ml_dtypes

import concourse.bass as bass
import concourse.mybir as mybir
import concourse.tile as tile
from concourse.bass_utils import run_bass_kernel_spmd
from concourse.tile import add_dep_helper

N, M, P = 4096, 4096, 32
NCORES = 8
NSHARD = N // NCORES  # 512 rows of x1 per core
IBLK = 128            # output row-block = PSUM partition dim
JBLK = 512            # matmul free dim = one fp32 PSUM bank
PSW = 2048            # PSUM tile width (4 banks) = one exp-activation
NSTRIP = 4            # concurrent PE row strips (K=32 each)
NI = NSHARD // IBLK   # 4 row-blocks
NH = M // PSW         # 2 PSUM tiles per row-block

BF16 = mybir.dt.bfloat16
NPBF16 = ml_dtypes.bfloat16

# load1 column layout (all bf16, 128 partitions):
#   [A_hi 512 | A_lo 512 | Bh(h=0) 512 | Bl(h=0) 512 | Yn(h=0) 512 |
#    ones 128 | xn_bits 8 | pad 56]                           -> 2752 cols
# load2: [Bh(h=1) 512 | Bl(h=1) 512 | Yn(h=1) 512 | pad 64]   -> 1600 cols
AHI_O = 0
ALO_O = 512
BH0_O = 1024
BL0_O = 1536
YN0_O = 2048
ONES_O = 2560
XN_O = 2688
L1_W = 2752
L2_W = 1600


def _build_nc():
    nc = bass.Bass()
    l1_t = nc.declare_dram_parameter("l1_t", [IBLK, L1_W], BF16, isOutput=False)
    l2_t = nc.declare_dram_parameter("l2_t", [IBLK, L2_W], BF16, isOutput=False)
    out = nc.declare_dram_parameter("out", [NSHARD, M], mybir.dt.float32, isOutput=True)

    with tile.TileContext(nc) as tc:
        with (
            tc.tile_pool(name="inp", bufs=1) as inp_pool,
            tc.tile_pool(name="stage", bufs=1) as stage_pool,
            tc.tile_pool(name="ps", bufs=1, space="PSUM") as ps_pool,
        ):
            dma_insts = []
            l1_sb = inp_pool.tile([IBLK, L1_W], BF16, tag="l1")
            dma_insts.append(nc.sync.dma_start(out=l1_sb, in_=l1_t[:, :]))
            l2_sb = inp_pool.tile([IBLK, L2_W], BF16, tag="l2")
            dma_insts.append(nc.scalar.dma_start(out=l2_sb, in_=l2_t[:, :]))

            def rows(s, k=32):
                return slice(32 * s, 32 * s + k)

            def bh(h, s):
                sb, o = (l1_sb, BH0_O) if h == 0 else (l2_sb, 0)
                return sb[rows(s), o : o + JBLK]

            def bl(h, s):
                sb, o = (l1_sb, BL0_O) if h == 0 else (l2_sb, JBLK)
                return sb[rows(s), o : o + JBLK]

            def ynr(h, s):
                sb, o = (l1_sb, YN0_O) if h == 0 else (l2_sb, 2 * JBLK)
                return sb[rows(s, 3), o : o + JBLK]

            xn_bias = l1_sb[:, XN_O : XN_O + 2 * NI].bitcast(mybir.dt.float32)

            # Tiny ACT-engine read of l1 so the scalar engine observes the l1
            # DMA semaphore here (1 wait); the real activations then carry
            # only their PE wait (walrus rejects multi-wait ACTIVATE, and
            # Tile doesn't track that the PE wait transitively covers l1).
            scratch = inp_pool.tile([IBLK, 1], mybir.dt.float32, tag="scr")
            nc.scalar.copy(out=scratch, in_=l1_sb[:, 0:1])

            ps_tiles = [
                ps_pool.tile(
                    [IBLK, PSW], mybir.dt.float32, tag=f"ps{h}", name=f"ps{h}"
                )
                for h in range(NH)
            ]

            act_insts = []
            mm_insts = []
            for i in range(NI):
                out_sb = stage_pool.tile(
                    [IBLK, M], mybir.dt.float32, tag=f"out{i}", name=f"out{i}"
                )
                for h in range(NH):
                    ps = ps_tiles[h]
                    # 4 passes x 4 strips; strip s = column slice q=s of the
                    # PSUM tile. Inner loop cycles strips so consecutive
                    # matmuls run in different row groups (concurrent).
                    for p in range(4):
                        start = p == 0
                        stop = p == 3
                        for s in range(NSTRIP):
                            if p == 0:
                                lhsT = l1_sb[rows(s), AHI_O + i * IBLK : AHI_O + (i + 1) * IBLK]
                                rhs = bh(h, s)
                            elif p == 1:
                                lhsT = l1_sb[rows(s), AHI_O + i * IBLK : AHI_O + (i + 1) * IBLK]
                                rhs = bl(h, s)
                            elif p == 2:
                                lhsT = l1_sb[rows(s), ALO_O + i * IBLK : ALO_O + (i + 1) * IBLK]
                                rhs = bh(h, s)
                            else:
                                lhsT = l1_sb[rows(s, 3), ONES_O : ONES_O + IBLK]
                                rhs = ynr(h, s)
                            mm_insts.append(
                                nc.tensor.matmul(
                                    ps[:, s * JBLK : (s + 1) * JBLK],
                                    lhsT=lhsT,
                                    rhs=rhs,
                                    start=start,
                                    stop=stop,
                                    tile_position=(32 * s, 0),
                                )
                            )
                    act_insts.append(
                        nc.scalar.activation(
                            out=out_sb[:, h * PSW : (h + 1) * PSW],
                            in_=ps,
                            func=mybir.ActivationFunctionType.Exp,
                            bias=xn_bias[:, i : i + 1],
                            scale=1.0,
                        )
                    )
                    # Last two row-blocks: store each PSW half as soon as its
                    # activation lands (the early blocks' full-width stores
                    # already overlap compute; the tail ones wouldn't). Total
                    # DMA count stays at 8 = one per HWDGE lane.
                    if i >= NI - 2:
                        eng = nc.sync if (i + h) % 2 == 0 else nc.scalar
                        dma_insts.append(
                            eng.dma_start(
                                out=out[
                                    i * IBLK : (i + 1) * IBLK,
                                    h * PSW : (h + 1) * PSW,
                                ],
                                in_=out_sb[:, h * PSW : (h + 1) * PSW],
                            )
                        )
                if i < NI - 2:
                    eng = nc.sync if i % 2 == 0 else nc.scalar
                    dma_insts.append(
                        eng.dma_start(
                            out=out[i * IBLK : (i + 1) * IBLK, :], in_=out_sb
                        )
                    )

            # Demote ACT->ACT pseudo-deps (PSUM bank read-read serialization,
            # already ordered through the interleaved matmuls + same-engine
            # FIFO) to nosync: walrus rejects multi-wait ACTIVATE.
            import bass_rust as _br

            act_names = {a.ins.name for a in act_insts}
            for a in act_insts:
                deps = list(a.ins.sync_dependency_names())
                spurious = [d for d in deps if d in act_names]
                if spurious:
                    keep = [d for d in deps if d not in act_names]
                    a.ins.take_sync_dependencies()
                    a.ins.set_sync_dependencies(
                        _br.InstructionNameOrderedSet(keep)
                    )
                    a.ins.add_nosync_dependencies_from(
                        _br.InstructionNameOrderedSet(spurious)
                    )

            # Wait-funnel so the framework's kernel-tail drain needs no waits
            # of its own (walrus rejects its usual all-sems wait list).
            for t in [mm_insts[-1], act_insts[-1], *dma_insts]:
                nop = nc.sync.nop(nofuse=True, hint="tail_funnel")
                add_dep_helper(nop.ins, t.ins, True, "tail wait funnel")
    return nc


def _bf16_splits(x, n):
    """Split fp32 array into n bf16 parts summing to ~x."""
    parts = []
    rem = x.astype(np.float32)
    for _ in range(n):
        p = rem.astype(NPBF16)
        parts.append(p)
        rem = rem - p.astype(np.float32)
    return parts


def run(x1, x2, log_rho, log_sigma, trace=False):
    """Returns (K, exec_time_ns). exec_time_ns is None unless trace=True."""
    x1 = np.asarray(x1, dtype=np.float32)
    x2 = np.asarray(x2, dtype=np.float32)
    rho = float(np.exp(np.float64(np.asarray(log_rho))))
    log_sig = 2.0 * float(np.asarray(log_sigma))  # log(sigma)

    xs = (x1 / np.float32(rho)).astype(np.float32)
    ys = (x2 / np.float32(rho)).astype(np.float32)
    xn = np.einsum("np,np->n", xs, xs, dtype=np.float64)
    yn = np.einsum("mp,mp->m", ys, ys, dtype=np.float64)

    a = xs.T.astype(np.float32)  # (32, N)
    b = ys.T.astype(np.float32)  # (32, M)
    a_hi, a_lo = _bf16_splits(a, 2)
    b_hi, b_lo = _bf16_splits(b, 2)
    y1, y2, y3 = _bf16_splits((-0.5 * yn).astype(np.float32), 3)
    # per-row ACT bias: -0.5*||x_i||^2 + log(sigma), exact fp32
    xbias = ((-0.5 * xn) + log_sig).astype(np.float32)

    def pack_b(src, h):
        # strip s rows hold the columns strip s consumes: B[:, h*PSW+s*JBLK..]
        o = np.zeros((IBLK, JBLK), NPBF16)
        for s in range(NSTRIP):
            o[32 * s : 32 * s + 32] = src[:, h * PSW + s * JBLK : h * PSW + (s + 1) * JBLK]
        return o

    def pack_yn(h):
        o = np.zeros((IBLK, JBLK), NPBF16)
        for s in range(NSTRIP):
            for r, yr in enumerate((y1, y2, y3)):
                o[32 * s + r] = yr[h * PSW + s * JBLK : h * PSW + (s + 1) * JBLK]
        return o

    ones = np.zeros((IBLK, IBLK), NPBF16)
    for s in range(NSTRIP):
        ones[32 * s : 32 * s + 3] = NPBF16(1.0)

    l2 = np.zeros((IBLK, L2_W), NPBF16)
    l2[:, 0:JBLK] = pack_b(b_hi, 1)
    l2[:, JBLK : 2 * JBLK] = pack_b(b_lo, 1)
    l2[:, 2 * JBLK : 3 * JBLK] = pack_yn(1)

    nc = _build_nc()
    in_maps = []
    for c in range(NCORES):
        sl = slice(c * NSHARD, (c + 1) * NSHARD)
        l1 = np.zeros((IBLK, L1_W), NPBF16)
        for s in range(NSTRIP):
            l1[32 * s : 32 * s + 32, AHI_O : AHI_O + NSHARD] = a_hi[:, sl]
            l1[32 * s : 32 * s + 32, ALO_O : ALO_O + NSHARD] = a_lo[:, sl]
        l1[:, BH0_O : BH0_O + JBLK] = pack_b(b_hi, 0)
        l1[:, BL0_O : BL0_O + JBLK] = pack_b(b_lo, 0)
        l1[:, YN0_O : YN0_O + JBLK] = pack_yn(0)
        l1[:, ONES_O : ONES_O + IBLK] = ones
        # fp32 bias bits ride along as bf16 column pairs
        xb = np.zeros((IBLK, NI), np.float32)
        for i in range(NI):
            xb[:, i] = xbias[c * NSHARD + i * IBLK : c * NSHARD + (i + 1) * IBLK]
        l1[:, XN_O : XN_O + 2 * NI] = xb.view(np.uint16).view(NPBF16)
        in_maps.append({"l1_t": np.ascontiguousarray(l1), "l2_t": l2})

    res = run_bass_kernel_spmd(
        nc, in_maps, core_ids=list(range(NCORES)), trace=trace
    )
    full = np.concatenate(
        [res.results[c]["out"] for c in range(NCORES)], axis=0
    )
    return full, res.exec_time_ns


def kernel(x1, x2, log_rho, log_sigma):
    out, _ = run(x1, x2, log_rho, log_sigma, trace=False)
    return out


# revision 38
# speedup vs baseline: 1.5762x; 1.0683x over previous
"""Exponentiated-quadratic (RBF) kernel matrix on 8 Trainium2 NeuronCores.

K[i, j] = sigma * exp(-0.5 * ||x1_i/rho - x2_j/rho||^2)
        with sigma = exp(log_sigma)^2, rho = exp(log_rho)

Strategy
--------
Row-shard x1 across the 8 cores (512 rows each), replicate x2. Each core
computes S = (x1/rho) @ (x2/rho)^T - 0.5*||y_j||^2 on the tensor engine and
finishes with one ScalarE activation per PSUM tile:
K = exp(S + (-0.5*||x_i||^2 + 2*log_sigma)), using ACT's free per-partition
bias (exact fp32 for the x-norms) — so the whole epilogue is a single pass.

Matmul precision: 3-pass bf16 split (Ah.Bh + Ah.Bl + Al.Bh, fp32 PSUM
accumulation, the dropped Al.Bl term is ~2^-18 relative) plus a K=3
ones-weighted pass adding the triple-bf16-split -0.5*||y_j||^2 row. Measured
~2e-5 scale-relative output error.

PE utilisation: K=32 fits a 32-row strip of the 128x128 array, so four
matmul streams run CONCURRENTLY via tile_position=(32s, 0) — column slice q
of each PSUM tile runs in strip q. This quadruples matmul throughput and
lets each strip's LDWEIGHTS prefetch under the other strips' matmuls.

The 4-strip layout also makes every input DMA a dense 128-partition
transfer (the fast path: ~300 GB/s vs ~50 GB/s for a 34-partition load): B
is packed on the host so strip s's rows hold exactly the columns strip s
consumes (no duplication), A is replicated per strip, and the fp32 ACT bias
rides along bit-cast as bf16 column pairs. Two input loads go on the two
parallel HWDGE rings (sync + scalar); output stores alternate between the
rings as well.

walrus in this container rejects instructions carrying more than one
semaphore wait, which shapes several things: single fused input tensors
(first matmul = one wait), persistent PSUM tiles (pool re-allocation adds a
same-engine PE wait), total DMA count <= 8 (HWDGE lane reuse adds an
ordering wait), ACT->ACT pseudo-deps demoted to nosync, and a chain of
single-wait NOPs on the sync sequencer that "observes" every completion
before the framework's kernel-tail drain.
"""

import numpy as np
import ml_dtypes

import concourse.bass as bass
import concourse.mybir as mybir
import concourse.tile as tile
from concourse.bass_utils import run_bass_kernel_spmd
from concourse.tile import add_dep_helper

N, M, P = 4096, 4096, 32
NCORES = 8
NSHARD = N // NCORES  # 512 rows of x1 per core
IBLK = 128            # output row-block = PSUM partition dim
JBLK = 512            # matmul free dim = one fp32 PSUM bank
PSW = 2048            # PSUM tile width (4 banks) = one exp-activation
NSTRIP = 4            # concurrent PE row strips (K=32 each)
NI = NSHARD // IBLK   # 4 row-blocks
NH = M // PSW         # 2 PSUM tiles per row-block

BF16 = mybir.dt.bfloat16
NPBF16 = ml_dtypes.bfloat16

# load1 column layout (all bf16, 128 partitions):
#   [A_hi 512 | A_lo 512 | Bh(h=0) 512 | Bl(h=0) 512 | Yn(h=0) 512 |
#    ones 128 | xn_bits 8 | pad 56]                           -> 2752 cols
# load2: [Bh(h=1) 512 | Bl(h=1) 512 | Yn(h=1) 512 | pad 64]   -> 1600 cols
AHI_O = 0
ALO_O = 512
BH0_O = 1024
BL0_O = 1536
YN0_O = 2048
ONES_O = 2560
XN_O = 2688
L1_W = 2752
L2_W = 1600


def _build_nc():
    nc = bass.Bass()
    l1_t = nc.declare_dram_parameter("l1_t", [IBLK, L1_W], BF16, isOutput=False)
    l2_t = nc.declare_dram_parameter("l2_t", [IBLK, L2_W], BF16, isOutput=False)
    out = nc.declare_dram_parameter("out", [NSHARD, M], mybir.dt.float32, isOutput=True)

    with tile.TileContext(nc) as tc:
        with (
            tc.tile_pool(name="inp", bufs=1) as inp_pool,
            tc.tile_pool(name="stage", bufs=1) as stage_pool,
            tc.tile_pool(name="ps", bufs=1, space="PSUM") as ps_pool,
        ):
            dma_insts = []
            l1_sb = inp_pool.tile([IBLK, L1_W], BF16, tag="l1")
            dma_insts.append(nc.sync.dma_start(out=l1_sb, in_=l1_t[:, :]))
            l2_sb = inp_pool.tile([IBLK, L2_W], BF16, tag="l2")
            dma_insts.append(nc.scalar.dma_start(out=l2_sb, in_=l2_t[:, :]))

            def rows(s, k=32):
                return slice(32 * s, 32 * s + k)

            def bh(h, s):
                sb, o = (l1_sb, BH0_O) if h == 0 else (l2_sb, 0)
                return sb[rows(s), o : o + JBLK]

            def bl(h, s):
                sb, o = (l1_sb, BL0_O) if h == 0 else (l2_sb, JBLK)
                return sb[rows(s), o : o + JBLK]

            def ynr(h, s):
                sb, o = (l1_sb, YN0_O) if h == 0 else (l2_sb, 2 * JBLK)
                return sb[rows(s, 3), o : o + JBLK]

            xn_bias = l1_sb[:, XN_O : XN_O + 2 * NI].bitcast(mybir.dt.float32)

            # Tiny ACT-engine read of l1 so the scalar engine observes the l1
            # DMA semaphore here (1 wait); the real activations then carry
            # only their PE wait (walrus rejects multi-wait ACTIVATE, and
            # Tile doesn't track that the PE wait transitively covers l1).
            scratch = inp_pool.tile([IBLK, 1], mybir.dt.float32, tag="scr")
            nc.scalar.copy(out=scratch, in_=l1_sb[:, 0:1])

            ps_tiles = [
                ps_pool.tile(
                    [IBLK, PSW], mybir.dt.float32, tag=f"ps{h}", name=f"ps{h}"
                )
                for h in range(NH)
            ]

            act_insts = []
            mm_insts = []
            for i in range(NI):
                out_sb = stage_pool.tile(
                    [IBLK, M], mybir.dt.float32, tag=f"out{i}", name=f"out{i}"
                )
                for h in range(NH):
                    ps = ps_tiles[h]
                    # 4 passes x 4 strips; strip s = column slice q=s of the
                    # PSUM tile. Inner loop cycles strips so consecutive
                    # matmuls run in different row groups (concurrent).
                    for p in range(4):
                        start = p == 0
                        stop = p == 3
                        for s in range(NSTRIP):
                            if p == 0:
                                lhsT = l1_sb[rows(s), AHI_O + i * IBLK : AHI_O + (i + 1) * IBLK]
                                rhs = bh(h, s)
                            elif p == 1:
                                lhsT = l1_sb[rows(s), AHI_O + i * IBLK : AHI_O + (i + 1) * IBLK]
                                rhs = bl(h, s)
                            elif p == 2:
                                lhsT = l1_sb[rows(s), ALO_O + i * IBLK : ALO_O + (i + 1) * IBLK]
                                rhs = bh(h, s)
                            else:
                                lhsT = l1_sb[rows(s, 3), ONES_O : ONES_O + IBLK]
                                rhs = ynr(h, s)
                            mm_insts.append(
                                nc.tensor.matmul(
                                    ps[:, s * JBLK : (s + 1) * JBLK],
                                    lhsT=lhsT,
                                    rhs=rhs,
                                    start=start,
                                    stop=stop,
                                    tile_position=(32 * s, 0),
                                )
                            )
                    act_insts.append(
                        nc.scalar.activation(
                            out=out_sb[:, h * PSW : (h + 1) * PSW],
                            in_=ps,
                            func=mybir.ActivationFunctionType.Exp,
                            bias=xn_bias[:, i : i + 1],
                            scale=1.0,
                        )
                    )
                    # Last two row-blocks: store each PSW half as soon as its
                    # activation lands (the early blocks' full-width stores
                    # already overlap compute; the tail ones wouldn't). Total
                    # DMA count stays at 8 = one per HWDGE lane.
                    if i >= NI - 2:
                        eng = nc.sync if (i + h) % 2 == 0 else nc.scalar
                        dma_insts.append(
                            eng.dma_start(
                                out=out[
                                    i * IBLK : (i + 1) * IBLK,
                                    h * PSW : (h + 1) * PSW,
                                ],
                                in_=out_sb[:, h * PSW : (h + 1) * PSW],
                            )
                        )
                if i < NI - 2:
                    eng = nc.sync if i % 2 == 0 else nc.scalar
                    dma_insts.append(
                        eng.dma_start(
                            out=out[i * IBLK : (i + 1) * IBLK, :], in_=out_sb
                        )
                    )

            # Demote ACT->ACT pseudo-deps (PSUM bank read-read serialization,
            # already ordered through the interleaved matmuls + same-engine
            # FIFO) to nosync: walrus rejects multi-wait ACTIVATE.
            import bass_rust as _br

            act_names = {a.ins.name for a in act_insts}
            for a in act_insts:
                deps = list(a.ins.sync_dependency_names())
                spurious = [d for d in deps if d in act_names]
                if spurious:
                    keep = [d for d in deps if d not in act_names]
                    a.ins.take_sync_dependencies()
                    a.ins.set_sync_dependencies(
                        _br.InstructionNameOrderedSet(keep)
                    )
                    a.ins.add_nosync_dependencies_from(
                        _br.InstructionNameOrderedSet(spurious)
                    )

            # Wait-funnel so the framework's kernel-tail drain needs no waits
            # of its own (walrus rejects its usual all-sems wait list). Each
            # nop also nosync-orders after every DMA so the scheduler cannot
            # slot a slow-waiting nop ahead of a still-pending store on the
            # same queue (head-of-line blocking).
            for t in [mm_insts[-1], act_insts[-1], *dma_insts]:
                nop = nc.sync.nop(nofuse=True, hint="tail_funnel")
                add_dep_helper(nop.ins, t.ins, True, "tail wait funnel")
                for dd in dma_insts:
                    if dd is not t:
                        add_dep_helper(nop.ins, dd.ins, False, "funnel order")
    return nc


def _bf16_splits(x, n):
    """Split fp32 array into n bf16 parts summing to ~x."""
    parts = []
    rem = x.astype(np.float32)
    for _ in range(n):
        p = rem.astype(NPBF16)
        parts.append(p)
        rem = rem - p.astype(np.float32)
    return parts


def run(x1, x2, log_rho, log_sigma, trace=False):
    """Returns (K, exec_time_ns). exec_time_ns is None unless trace=True."""
    x1 = np.asarray(x1, dtype=np.float32)
    x2 = np.asarray(x2, dtype=np.float32)
    rho = float(np.exp(np.float64(np.asarray(log_rho))))
    log_sig = 2.0 * float(np.asarray(log_sigma))  # log(sigma)

    xs = (x1 / np.float32(rho)).astype(np.float32)
    ys = (x2 / np.float32(rho)).astype(np.float32)
    xn = np.einsum("np,np->n", xs, xs, dtype=np.float64)
    yn = np.einsum("mp,mp->m", ys, ys, dtype=np.float64)

    a = xs.T.astype(np.float32)  # (32, N)
    b = ys.T.astype(np.float32)  # (32, M)
    a_hi, a_lo = _bf16_splits(a, 2)
    b_hi, b_lo = _bf16_splits(b, 2)
    y1, y2, y3 = _bf16_splits((-0.5 * yn).astype(np.float32), 3)
    # per-row ACT bias: -0.5*||x_i||^2 + log(sigma), exact fp32
    xbias = ((-0.5 * xn) + log_sig).astype(np.float32)

    def pack_b(src, h):
        # strip s rows hold the columns strip s consumes: B[:, h*PSW+s*JBLK..]
        o = np.zeros((IBLK, JBLK), NPBF16)
        for s in range(NSTRIP):
            o[32 * s : 32 * s + 32] = src[:, h * PSW + s * JBLK : h * PSW + (s + 1) * JBLK]
        return o

    def pack_yn(h):
        o = np.zeros((IBLK, JBLK), NPBF16)
        for s in range(NSTRIP):
            for r, yr in enumerate((y1, y2, y3)):
                o[32 * s + r] = yr[h * PSW + s * JBLK : h * PSW + (s + 1) * JBLK]
        return o

    ones = np.zeros((IBLK, IBLK), NPBF16)
    for s in range(NSTRIP):
        ones[32 * s : 32 * s + 3] = NPBF16(1.0)

    l2 = np.zeros((IBLK, L2_W), NPBF16)
    l2[:, 0:JBLK] = pack_b(b_hi, 1)
    l2[:, JBLK : 2 * JBLK] = pack_b(b_lo, 1)
    l2[:, 2 * JBLK : 3 * JBLK] = pack_yn(1)

    nc = _build_nc()
    in_maps = []
    for c in range(NCORES):
        sl = slice(c * NSHARD, (c + 1) * NSHARD)
        l1 = np.zeros((IBLK, L1_W), NPBF16)
        for s in range(NSTRIP):
            l1[32 * s : 32 * s + 32, AHI_O : AHI_O + NSHARD] = a_hi[:, sl]
            l1[32 * s : 32 * s + 32, ALO_O : ALO_O + NSHARD] = a_lo[:, sl]
        l1[:, BH0_O : BH0_O + JBLK] = pack_b(b_hi, 0)
        l1[:, BL0_O : BL0_O + JBLK] = pack_b(b_lo, 0)
        l1[:, YN0_O : YN0_O + JBLK] = pack_yn(0)
        l1[:, ONES_O : ONES_O + IBLK] = ones
        # fp32 bias bits ride along as bf16 column pairs
        xb = np.zeros((IBLK, NI), np.float32)
        for i in range(NI):
            xb[:, i] = xbias[c * NSHARD + i * IBLK : c * NSHARD + (i + 1) * IBLK]
        l1[:, XN_O : XN_O + 2 * NI] = xb.view(np.uint16).view(NPBF16)
        in_maps.append({"l1_t": np.ascontiguousarray(l1), "l2_t": l2})

    res = run_bass_kernel_spmd(
        nc, in_maps, core_ids=list(range(NCORES)), trace=trace
    )
    full = np.concatenate(
        [res.results[c]["out"] for c in range(NCORES)], axis=0
    )
    return full, res.exec_time_ns


def kernel(x1, x2, log_rho, log_sigma):
    out, _ = run(x1, x2, log_rho, log_sigma, trace=False)
    return out
